# revision 12
# baseline (speedup 1.0000x reference)
"""Trainium2 Bass kernel for nn_AttentionBlock (GroupNorm + MHA + residual).

Sharding: data-parallel over batch. 8 batch elements -> 8 NeuronCores.
Each core runs the full attention block for one (512, 1024) slice.

Shapes (per core):
  x:  (C=512, L=1024) f32, laid out in SBUF as (128, 4, 1024) [c%128, c//128, l]
  GroupNorm: 32 groups of 16 channels over all L.
  q = Wq@xn + bq ; k,v = Wkv@xn + bkv ; heads of dim 64
  dots_T[s,t] = sum_d k[d,s] q[d,t]  (psum, per head, s-tile on partitions)
  p = exp(dots/64) (bf16) ; out = (p-weighted v) / rowsum ; y = Wo@out + bo + x
"""

import numpy as np

import concourse.bass as bass
import concourse.bacc as bacc_mod
import concourse.mybir as mybir
import concourse.tile as tile

P = 128
CT = 4          # channel tiles (512 = 4*128)
C = 512
L = 1024
NH = 8
DH = 64
G = 32
GS = 16         # channels per group
EPS = 1e-5
ST = 8          # s tiles (1024 = 8*128)
TH = 2          # t halves (1024 = 2*512)
F32 = mybir.dt.float32
BF16 = mybir.dt.bfloat16
I32 = mybir.dt.int32
AF = mybir.ActivationFunctionType
ALU = mybir.AluOpType

NP_BF16 = mybir.dt.np(BF16)


def build_nc(debug: bool = False) -> bass.Bass:
    nc = bacc_mod.Bacc()

    x_d = nc.declare_dram_parameter("x", [P, CT, L], F32, isOutput=False)
    wqt_d = nc.declare_dram_parameter("wqt", [P, CT, C], BF16, isOutput=False)
    wkt_d = nc.declare_dram_parameter("wkt", [P, CT, C], BF16, isOutput=False)
    wvt_d = nc.declare_dram_parameter("wvt", [P, CT, C], BF16, isOutput=False)
    wot_d = nc.declare_dram_parameter("wot", [P, CT, C], BF16, isOutput=False)
    bq_d = nc.declare_dram_parameter("bq", [P, CT], F32, isOutput=False)
    bk_d = nc.declare_dram_parameter("bk", [P, CT], F32, isOutput=False)
    bv_d = nc.declare_dram_parameter("bv", [1, C], BF16, isOutput=False)
    bo_d = nc.declare_dram_parameter("bo", [1, C], BF16, isOutput=False)
    gam_d = nc.declare_dram_parameter("gamma", [P, CT], F32, isOutput=False)
    bet_d = nc.declare_dram_parameter("beta", [P, CT], F32, isOutput=False)
    gsel_d = nc.declare_dram_parameter("gsel", [P, CT, G], F32, isOutput=False)
    gbc_d = nc.declare_dram_parameter("gbc", [G, CT, P], F32, isOutput=False)
    y_d = nc.declare_dram_parameter("y", [P, CT, L], F32, isOutput=True)
    if debug:
        dbg_stats_d = nc.declare_dram_parameter("dbg_stats", [G, 2], F32, isOutput=True)
        dbg_xn_d = nc.declare_dram_parameter("dbg_xn", [P, CT, L], BF16, isOutput=True)
        dbg_q_d = nc.declare_dram_parameter("dbg_q", [P, CT, L], BF16, isOutput=True)
        dbg_k_d = nc.declare_dram_parameter("dbg_k", [P, CT, L], BF16, isOutput=True)
        dbg_vt_d = nc.declare_dram_parameter("dbg_vt", [P, ST, NH * 65], BF16, isOutput=True)
        dbg_p_d = nc.declare_dram_parameter("dbg_p", [2, P, ST, L], BF16, isOutput=True)
        dbg_rs_d = nc.declare_dram_parameter("dbg_rs", [33, 512], F32, isOutput=True)
        dbg_attn_d = nc.declare_dram_parameter("dbg_attn", [P, CT, L], BF16, isOutput=True)
        dbg_bc_d = nc.declare_dram_parameter("dbg_bc", [P, 512], F32, isOutput=True)
        dbg_pav_d = nc.declare_dram_parameter("dbg_pav", [P, 512], F32, isOutput=True)

    with tile.TileContext(nc) as tc:
        with (
            tc.tile_pool(name="big", bufs=1) as big,
            tc.tile_pool(name="pbuf", bufs=4) as pbuf,
            tc.tile_pool(name="work", bufs=4) as work,
            tc.tile_pool(name="scal", bufs=4) as scal,
            tc.tile_pool(name="bcp", bufs=3) as bcp,
            tc.tile_pool(name="yp", bufs=3) as yp,
            tc.tile_pool(name="ps", bufs=2, space="PSUM") as psp,
            tc.tile_pool(name="dscr", bufs=4, space="DRAM") as dscr,
        ):
            _psn = [0]

            def psum_tile():
                _psn[0] += 1
                return psp.tile(
                    [P, 2, 512], F32, tag="ps", name=f"pst{_psn[0]}"
                )

            # ---- constants / inputs ----
            ones_1_128 = big.tile([1, 128], BF16)
            nc.vector.memset(ones_1_128, 1.0)
            ones_1_512 = big.tile([1, 512], BF16)
            nc.vector.memset(ones_1_512, 1.0)
            ones_128_1 = big.tile([P, 1], BF16)
            nc.vector.memset(ones_128_1, 1.0)

            x_sb = big.tile([P, CT, L], F32)
            for t in range(CT):
                nc.sync.dma_start(out=x_sb[:, t, :], in_=x_d[:, t, :])

            wqt_sb = big.tile([P, CT, C], BF16)
            nc.sync.dma_start(out=wqt_sb, in_=wqt_d[:])
            wkt_sb = big.tile([P, CT, C], BF16)
            nc.sync.dma_start(out=wkt_sb, in_=wkt_d[:])
            wvt_sb = big.tile([P, CT, C], BF16)
            nc.sync.dma_start(out=wvt_sb, in_=wvt_d[:])
            wot_sb = big.tile([P, CT, C], BF16)
            nc.sync.dma_start(out=wot_sb, in_=wot_d[:])
            bq_sb = big.tile([P, CT], F32)
            nc.sync.dma_start(out=bq_sb, in_=bq_d[:])
            bk_sb = big.tile([P, CT], F32)
            nc.sync.dma_start(out=bk_sb, in_=bk_d[:])
            bv_sb = big.tile([1, C], BF16)
            nc.sync.dma_start(out=bv_sb, in_=bv_d[:])
            bo_sb = big.tile([1, C], BF16)
            nc.sync.dma_start(out=bo_sb, in_=bo_d[:])
            gam_sb = big.tile([P, CT], F32)
            nc.sync.dma_start(out=gam_sb, in_=gam_d[:])
            bet_sb = big.tile([P, CT], F32)
            nc.sync.dma_start(out=bet_sb, in_=bet_d[:])
            gsel_sb = big.tile([P, CT, G], F32)
            nc.sync.dma_start(out=gsel_sb, in_=gsel_d[:])
            gbc_sb = big.tile([G, CT, P], F32)
            nc.sync.dma_start(out=gbc_sb, in_=gbc_d[:])

            # Trigger the exp table load early so it is off the critical path.
            dummy = big.tile([1, 1], F32)
            nc.scalar.activation(out=dummy, in_=ones_1_128[0:1, 0:1], func=AF.Exp)

            # ---- GroupNorm statistics ----
            # per-channel mean/var via bn_stats, then group-aggregate on PE.
            psg = psum_tile()  # [0:32, 0, 0:2] = [mean_g, E2_g]
            for t in range(CT):
                st6 = work.tile([P, 2, 6], F32, tag="st6")
                for j in range(2):
                    nc.vector.bn_stats(
                        out=st6[:, j, :], in_=x_sb[:, t, 512 * j:512 * (j + 1)]
                    )
                mv = work.tile([P, 2], F32, tag="mv")
                nc.vector.bn_aggr(out=mv, in_=st6)
                # rhs2 = [mean_c, var_c + mean_c^2]
                sq = work.tile([P, 1], F32, tag="sq")
                nc.vector.tensor_mul(sq, mv[:, 0:1], mv[:, 0:1])
                rhs2 = work.tile([P, 2], F32, tag="rhs2")
                nc.vector.tensor_copy(rhs2[:, 0:1], mv[:, 0:1])
                nc.vector.tensor_add(rhs2[:, 1:2], mv[:, 1:2], sq)
                nc.tensor.matmul(
                    psg[0:G, 0, 0:2],
                    lhsT=gsel_sb[:, t, :],
                    rhs=rhs2,
                    start=(t == 0),
                    stop=(t == CT - 1),
                )

            # stats2 = [mean_g, rstd_g] in SBUF (32, 2)
            stats2 = big.tile([G, 2], F32)
            nc.vector.tensor_copy(stats2[:, 0:1], psg[0:G, 0, 0:1])
            sqg = scal.tile([G, 1], F32, tag="sqg")
            nc.vector.tensor_mul(sqg, stats2[:, 0:1], stats2[:, 0:1])
            varg = scal.tile([G, 1], F32, tag="varg")
            nc.vector.tensor_sub(varg, psg[0:G, 0, 1:2], sqg)
            nc.vector.tensor_scalar(
                out=varg, in0=varg, scalar1=EPS, scalar2=None, op0=ALU.add
            )
            # rstd = rsqrt(varg) via bit-trick + 3 Newton iterations (all DVE)
            yv = scal.tile([G, 1], F32, tag="yv")
            tI = scal.tile([G, 1], I32, tag="tI")
            nc.vector.tensor_scalar(
                out=tI, in0=varg.bitcast(I32), scalar1=1, scalar2=None,
                op0=ALU.logical_shift_right,
            )
            nc.vector.tensor_scalar(
                out=yv.bitcast(I32), in0=tI, scalar1=-1, scalar2=0x5F3759DF,
                op0=ALU.mult, op1=ALU.add,
            )
            for _ in range(3):
                t1 = scal.tile([G, 1], F32, tag="t1")
                nc.vector.tensor_mul(t1, yv, yv)
                nc.vector.tensor_mul(t1, t1, varg)
                nc.vector.tensor_scalar(
                    out=t1, in0=t1, scalar1=-0.5, scalar2=1.5,
                    op0=ALU.mult, op1=ALU.add,
                )
                nc.vector.tensor_mul(yv, yv, t1)
            nc.vector.tensor_copy(stats2[:, 1:2], yv)

            # ---- normalize: xn = x*A + B (per channel) ----
            xn_sb = big.tile([P, CT, L], BF16)
            for t in range(CT):
                psb = psum_tile()
                nc.tensor.matmul(
                    psb[0:P, 0, 0:2], lhsT=gbc_sb[:, t, :], rhs=stats2,
                    start=True, stop=True,
                )
                a_t = scal.tile([P, 1], F32, tag="a_t")
                nc.vector.tensor_mul(a_t, psb[0:P, 0, 1:2], gam_sb[:, t:t + 1])
                tmp = scal.tile([P, 1], F32, tag="tmp")
                nc.vector.tensor_mul(tmp, psb[0:P, 0, 0:1], a_t)
                b_t = scal.tile([P, 1], F32, tag="b_t")
                nc.vector.tensor_sub(b_t, bet_sb[:, t:t + 1], tmp)
                nc.vector.tensor_scalar(
                    out=xn_sb[:, t, :], in0=x_sb[:, t, :],
                    scalar1=a_t, scalar2=b_t, op0=ALU.mult, op1=ALU.add,
                )

            if debug:
                nc.sync.dma_start(out=dbg_stats_d[:], in_=stats2)
                nc.sync.dma_start(out=dbg_xn_d[:], in_=xn_sb)

            # ---- projections ----
            q_sb = big.tile([P, CT, L], BF16)
            k_sb = big.tile([P, CT, L], BF16)
            for j in range(CT):          # output channel tile
                for h2 in range(TH):     # position half
                    pq = psum_tile()
                    for ct in range(CT):
                        nc.tensor.matmul(
                            pq[:, 0, :],
                            lhsT=wqt_sb[:, ct, 128 * j:128 * (j + 1)],
                            rhs=xn_sb[:, ct, 512 * h2:512 * (h2 + 1)],
                            start=(ct == 0), stop=(ct == CT - 1),
                        )
                    nc.vector.tensor_scalar(
                        out=q_sb[:, j, 512 * h2:512 * (h2 + 1)], in0=pq[:, 0, :],
                        scalar1=bq_sb[:, j:j + 1], scalar2=None, op0=ALU.add,
                    )
                    pk = psum_tile()
                    for ct in range(CT):
                        nc.tensor.matmul(
                            pk[:, 0, :],
                            lhsT=wkt_sb[:, ct, 128 * j:128 * (j + 1)],
                            rhs=xn_sb[:, ct, 512 * h2:512 * (h2 + 1)],
                            start=(ct == 0), stop=(ct == CT - 1),
                        )
                    nc.vector.tensor_scalar(
                        out=k_sb[:, j, 512 * h2:512 * (h2 + 1)], in0=pk[:, 0, :],
                        scalar1=bk_sb[:, j:j + 1], scalar2=None, op0=ALU.add,
                    )

            # vT: (l, o) layout with per-head ones column at slot 64 of 65.
            vT_sb = big.tile([P, ST, NH * 65], BF16)
            ones_cols = vT_sb.rearrange(
                "p s (h c) -> p s h c", c=65
            )[:, :, :, 64:65]
            nc.vector.memset(ones_cols, 1.0)
            for lt in range(ST):
                pv = psum_tile()
                for ct in range(CT):
                    nc.tensor.matmul(
                        pv[:, 0, :],
                        lhsT=xn_sb[:, ct, 128 * lt:128 * (lt + 1)],
                        rhs=wvt_sb[:, ct, :],
                        start=(ct == 0), stop=False,
                    )
                nc.tensor.matmul(
                    pv[:, 0, :], lhsT=ones_1_128, rhs=bv_sb,
                    start=False, stop=True,
                )
                dst = vT_sb.rearrange("p s (h c) -> p s h c", c=65)[:, lt, :, 0:64]
                src = pv[:, 0, :].rearrange("p (h c) -> p h c", c=64)
                nc.vector.tensor_copy(dst, src)

            if debug:
                nc.sync.dma_start(out=dbg_q_d[:], in_=q_sb)
                nc.sync.dma_start(out=dbg_k_d[:], in_=k_sb)
                nc.sync.dma_start(out=dbg_vt_d[:], in_=vT_sb)

            # ---- attention, head pairs (2pr, 2pr+1) live in c-tile pr ----
            attn_sb = big.tile([P, CT, L], BF16)
            vT_h = vT_sb.rearrange("p s (h c) -> p s h c", c=65)
            for pr in range(CT):
                pA = pbuf.tile([P, ST, L], BF16, tag="p")
                pB = pbuf.tile([P, ST, L], BF16, tag="p")
                for j in range(ST):
                    pdA = psum_tile()
                    pdB = psum_tile()
                    for th in range(TH):
                        nc.tensor.matmul(
                            pdA[:, th, :],
                            lhsT=k_sb[0:64, pr, 128 * j:128 * (j + 1)],
                            rhs=q_sb[0:64, pr, 512 * th:512 * (th + 1)],
                            start=True, stop=True,
                        )
                        nc.tensor.matmul(
                            pdB[:, th, :],
                            lhsT=k_sb[64:128, pr, 128 * j:128 * (j + 1)],
                            rhs=q_sb[64:128, pr, 512 * th:512 * (th + 1)],
                            start=True, stop=True,
                        )
                    nc.scalar.activation(
                        out=pA[:, j, :].rearrange("p (a b) -> p a b", a=2),
                        in_=pdA[:, :, :], func=AF.Exp, scale=1.0 / 64.0,
                    )
                    nc.scalar.activation(
                        out=pB[:, j, :].rearrange("p (a b) -> p a b", a=2),
                        in_=pdB[:, :, :], func=AF.Exp, scale=1.0 / 64.0,
                    )
                if debug and pr == 0:
                    nc.sync.dma_start(out=dbg_p_d[0], in_=pA)
                    nc.sync.dma_start(out=dbg_p_d[1], in_=pB)
                hA, hB = 2 * pr, 2 * pr + 1
                for th in range(TH):
                    # Separate PSUM banks per head so each accumulation group
                    # owns its bank (start=True clears the whole bank).  Col
                    # positions: vA@(0,0) vB@(0,64) run concurrently, then
                    # rowsums rsA@(0,0) rsB@(0,32) run concurrently.
                    pavA = psp.tile([P, 512], F32, tag="ps1", bufs=4, name=f"pavA{pr}{th}")
                    pavB = psp.tile([P, 512], F32, tag="ps1", bufs=4, name=f"pavB{pr}{th}")
                    prsA = psp.tile([33, 512], F32, tag="ps1", bufs=4, name=f"prsA{pr}{th}")
                    prsB = psp.tile([33, 512], F32, tag="ps1", bufs=4, name=f"prsB{pr}{th}")
                    tsl = slice(512 * th, 512 * (th + 1))
                    for j in range(ST):
                        st_, sp_ = (j == 0), (j == ST - 1)
                        nc.tensor.matmul(
                            pavA[0:64, :], lhsT=vT_h[:, j, hA, 0:64],
                            rhs=pA[:, j, tsl], start=st_, stop=sp_,
                        )
                        nc.tensor.matmul(
                            pavB[64:128, :], lhsT=vT_h[:, j, hB, 0:64],
                            rhs=pB[:, j, tsl], start=st_, stop=sp_,
                        )
                        nc.tensor.matmul(
                            prsA[0:1, :], lhsT=ones_128_1,
                            rhs=pA[:, j, tsl], start=st_, stop=sp_,
                        )
                        nc.tensor.matmul(
                            prsB[32:33, :], lhsT=ones_128_1,
                            rhs=pB[:, j, tsl], start=st_, stop=sp_,
                        )
                    rec = bcp.tile([33, 512], F32, tag="rec")
                    nc.vector.reciprocal(rec[0:1, :], prsA[0:1, :])
                    nc.vector.reciprocal(rec[32:33, :], prsB[32:33, :])
                    # Broadcast recA/recB across partitions by bouncing
                    # through DRAM (partition-0-step DMA only works from DRAM).
                    rdr = dscr.tile([2, 512], F32, tag="rdr", name=f"rdr{pr}{th}")
                    nc.sync.dma_start(out=rdr[0:1, :], in_=rec[0:1, :])
                    nc.sync.dma_start(out=rdr[1:2, :], in_=rec[32:33, :])
                    bc = bcp.tile([P, 512], F32, tag="bc")
                    nc.gpsimd.dma_start(
                        out=bc[0:64, :], in_=rdr[0, :].partition_broadcast(64)
                    )
                    nc.gpsimd.dma_start(
                        out=bc[64:128, :], in_=rdr[1, :].partition_broadcast(64)
                    )
                    if debug and pr == 0 and th == 0:
                        nc.sync.dma_start(out=dbg_rs_d[:], in_=rec)
                        nc.sync.dma_start(out=dbg_bc_d[:], in_=bc)
                        pavc = bcp.tile([P, 512], F32, tag="pavc")
                        nc.vector.tensor_copy(pavc[0:64, :], pavA[0:64, :])
                        nc.vector.tensor_copy(pavc[64:128, :], pavB[64:128, :])
                        nc.sync.dma_start(out=dbg_pav_d[:], in_=pavc)
                    nc.vector.tensor_mul(
                        attn_sb[0:64, pr, tsl], pavA[0:64, :], bc[0:64, :]
                    )
                    nc.vector.tensor_mul(
                        attn_sb[64:128, pr, tsl], pavB[64:128, :], bc[64:128, :]
                    )

            if debug:
                nc.sync.dma_start(out=dbg_attn_d[:], in_=attn_sb)

            # ---- output projection + residual ----
            for j in range(CT):
                for h2 in range(TH):
                    po = psum_tile()
                    for dt_ in range(CT):
                        nc.tensor.matmul(
                            po[:, 0, :],
                            lhsT=wot_sb[:, dt_, 128 * j:128 * (j + 1)],
                            rhs=attn_sb[:, dt_, 512 * h2:512 * (h2 + 1)],
                            start=(dt_ == 0), stop=False,
                        )
                    nc.tensor.matmul(
                        po[:, 0, :], lhsT=bo_sb[:, 128 * j:128 * (j + 1)],
                        rhs=ones_1_512, start=False, stop=True,
                    )
                    ytile = yp.tile([P, 512], F32, tag="y")
                    nc.vector.tensor_add(
                        ytile, po[:, 0, :], x_sb[:, j, 512 * h2:512 * (h2 + 1)]
                    )
                    nc.sync.dma_start(
                        out=y_d[:, j, 512 * h2:512 * (h2 + 1)], in_=ytile
                    )

    return nc


def _ctile(a):
    """(512, X) -> (128, 4, X) channel-tile layout."""
    return np.ascontiguousarray(
        a.reshape(4, 128, *a.shape[1:]).transpose(1, 0, *range(2, a.ndim + 1))
    )


def prep_consts(gamma, beta, Wq, bq, Wkv, bkv, Wo, bo):
    grp = np.arange(C) // GS
    gsel = (grp[:, None] == np.arange(G)[None, :]).astype(np.float32) / GS
    gbc = (np.arange(G)[:, None] == grp[None, :]).astype(np.float32)
    consts = {
        "wqt": _ctile(np.ascontiguousarray(Wq.T)).astype(NP_BF16),
        "wkt": _ctile(np.ascontiguousarray(Wkv[:C].T)).astype(NP_BF16),
        "wvt": _ctile(np.ascontiguousarray(Wkv[C:].T)).astype(NP_BF16),
        "wot": _ctile(np.ascontiguousarray(Wo.T)).astype(NP_BF16),
        "bq": np.ascontiguousarray(bq.reshape(4, 128).T).astype(np.float32),
        "bk": np.ascontiguousarray(bkv[:C].reshape(4, 128).T).astype(np.float32),
        "bv": bkv[C:].reshape(1, C).astype(NP_BF16),
        "bo": bo.reshape(1, C).astype(NP_BF16),
        "gamma": np.ascontiguousarray(gamma.reshape(4, 128).T).astype(np.float32),
        "beta": np.ascontiguousarray(beta.reshape(4, 128).T).astype(np.float32),
        "gsel": np.ascontiguousarray(gsel.reshape(4, 128, G).transpose(1, 0, 2)),
        "gbc": np.ascontiguousarray(gbc.reshape(G, 4, 128)),
    }
    return consts


def prep_x(x):
    """(8, 512, 32, 32) -> list of per-core (128, 4, 1024) f32."""
    xf = np.asarray(x, dtype=np.float32).reshape(8, C, L)
    return [_ctile(xf[i]) for i in range(8)]


def unprep_y(ys):
    """list of per-core (128, 4, 1024) -> (8, 512, 32, 32)."""
    out = np.empty((8, C, 32, 32), dtype=np.float32)
    for i, yi in enumerate(ys):
        out[i] = yi.transpose(1, 0, 2).reshape(C, 32, 32)
    return out


_NC_CACHE = None


def kernel(x, gamma, beta, Wq, bq, Wkv, bkv, Wo, bo):
    global _NC_CACHE
    from concourse.bass_utils import run_bass_kernel_spmd

    if _NC_CACHE is None:
        _NC_CACHE = build_nc()
        _NC_CACHE.finalize()
    nc = _NC_CACHE

    consts = prep_consts(
        np.asarray(gamma, np.float32), np.asarray(beta, np.float32),
        np.asarray(Wq, np.float32), np.asarray(bq, np.float32),
        np.asarray(Wkv, np.float32), np.asarray(bkv, np.float32),
        np.asarray(Wo, np.float32), np.asarray(bo, np.float32),
    )
    xs = prep_x(x)
    in_maps = [{**consts, "x": xs[i]} for i in range(8)]
    res = run_bass_kernel_spmd(nc, in_maps, core_ids=list(range(8)))
    return unprep_y([r["y"] for r in res.results])


# revision 28
# speedup vs baseline: 30.1572x; 30.1572x over previous
"""Trainium2 Bass kernel for nn_AttentionBlock (GroupNorm + MHA + residual).

Sharding: data-parallel over batch. 8 batch elements -> 8 NeuronCores.
Each core runs the full attention block for one (512, 1024) slice.

Shapes (per core):
  x:  (C=512, L=1024) f32, laid out in SBUF as (128, 4, 1024) [c%128, c//128, l]
  GroupNorm: 32 groups of 16 channels over all L.
  q = Wq@xn + bq ; k,v = Wkv@xn + bkv ; heads of dim 64
  dots_T[s,t] = sum_d k[d,s] q[d,t]  (psum, per head, s-tile on partitions)
  p = exp(dots/64) (bf16) ; out = (p-weighted v) / rowsum ; y = Wo@out + bo + x
"""

import numpy as np

import concourse.bass as bass
import concourse.bacc as bacc_mod
import concourse.mybir as mybir
import concourse.tile as tile

P = 128
CT = 4          # channel tiles (512 = 4*128)
C = 512
L = 1024
NH = 8
DH = 64
G = 32
GS = 16         # channels per group
EPS = 1e-5
ST = 8          # s tiles (1024 = 8*128)
TH = 2          # t halves (1024 = 2*512)
F32 = mybir.dt.float32
BF16 = mybir.dt.bfloat16
FP8 = mybir.dt.float8e4
LN128 = float(np.log(128.0))
I32 = mybir.dt.int32
AF = mybir.ActivationFunctionType
ALU = mybir.AluOpType

NP_BF16 = mybir.dt.np(BF16)
NP_FP8 = mybir.dt.np(FP8)


def build_nc(debug: bool = False) -> bass.Bass:
    nc = bacc_mod.Bacc()

    x_d = nc.declare_dram_parameter("x", [P, CT, L], F32, isOutput=False)
    wqt_d = nc.declare_dram_parameter("wqt", [P, CT, C], BF16, isOutput=False)
    wkt_d = nc.declare_dram_parameter("wkt", [P, CT, C], BF16, isOutput=False)
    wvt_d = nc.declare_dram_parameter("wvt", [P, CT, C], BF16, isOutput=False)
    wot_d = nc.declare_dram_parameter("wot", [DH, CT, 2, C], FP8, isOutput=False)
    bq_d = nc.declare_dram_parameter("bq", [P, CT], F32, isOutput=False)
    bk_d = nc.declare_dram_parameter("bk", [P, CT], F32, isOutput=False)
    bv_d = nc.declare_dram_parameter("bv", [1, C], BF16, isOutput=False)
    bo_d = nc.declare_dram_parameter("bo", [1, C], BF16, isOutput=False)
    gam_d = nc.declare_dram_parameter("gamma", [P, CT], F32, isOutput=False)
    bet_d = nc.declare_dram_parameter("beta", [P, CT], F32, isOutput=False)
    gsel_d = nc.declare_dram_parameter("gsel", [P, CT, G], F32, isOutput=False)
    gbc_d = nc.declare_dram_parameter("gbc", [G, CT, P], F32, isOutput=False)
    y_d = nc.declare_dram_parameter("y", [P, CT, L], F32, isOutput=True)
    if debug:
        dbg_stats_d = nc.declare_dram_parameter("dbg_stats", [G, 2], F32, isOutput=True)
        dbg_xn_d = nc.declare_dram_parameter("dbg_xn", [P, CT, L], BF16, isOutput=True)
        dbg_q_d = nc.declare_dram_parameter("dbg_q", [P, CT, L], BF16, isOutput=True)
        dbg_k_d = nc.declare_dram_parameter("dbg_k", [P, CT, L], BF16, isOutput=True)
        dbg_vt_d = nc.declare_dram_parameter("dbg_vt", [P, ST, NH * 80], FP8, isOutput=True)
        dbg_p_d = nc.declare_dram_parameter("dbg_p", [2, P, ST, L], FP8, isOutput=True)
        dbg_rs_d = nc.declare_dram_parameter("dbg_rs", [2, 512], F32, isOutput=True)
        dbg_attn_d = nc.declare_dram_parameter("dbg_attn", [DH, CT, 2, L], FP8, isOutput=True)
        dbg_pav_d = nc.declare_dram_parameter("dbg_pav", [P, 512], F32, isOutput=True)

    with tile.TileContext(nc) as tc:
        with (
            tc.tile_pool(name="big", bufs=1) as big,
            tc.tile_pool(name="pbuf", bufs=6) as pbuf,
            tc.tile_pool(name="work", bufs=4) as work,
            tc.tile_pool(name="scal", bufs=4) as scal,
            tc.tile_pool(name="bcp", bufs=3) as bcp,
            tc.tile_pool(name="yp", bufs=3) as yp,
            tc.tile_pool(name="ps", bufs=2, space="PSUM") as psp,
            tc.tile_pool(name="dscr", bufs=4, space="DRAM") as dscr,
        ):
            _psn = [0]

            def psum_tile():
                _psn[0] += 1
                return psp.tile(
                    [P, 2, 512], F32, tag="ps", name=f"pst{_psn[0]}"
                )

            # ---- constants / inputs ----
            ones_1_128 = big.tile([1, 128], BF16)
            nc.vector.memset(ones_1_128, 1.0)
            ones_1_512 = big.tile([1, 512], BF16)
            nc.vector.memset(ones_1_512, 1.0)
            ones_128_1 = big.tile([P, 1], BF16)
            nc.vector.memset(ones_128_1, 1.0)
            ln256_b = big.tile([P, 1], F32)
            nc.vector.memset(ln256_b, LN128)

            x_sb = big.tile([P, CT, L], F32)
            for t in range(CT):
                nc.sync.dma_start(out=x_sb[:, t, :], in_=x_d[:, t, :])

            wqt_sb = big.tile([P, CT, C], BF16)
            nc.sync.dma_start(out=wqt_sb, in_=wqt_d[:])
            wkt_sb = big.tile([P, CT, C], BF16)
            nc.sync.dma_start(out=wkt_sb, in_=wkt_d[:])
            wvt_sb = big.tile([P, CT, C], BF16)
            nc.sync.dma_start(out=wvt_sb, in_=wvt_d[:])
            wot_sb = big.tile([DH, CT, 2, C], FP8)
            nc.sync.dma_start(out=wot_sb, in_=wot_d[:])
            bq_sb = big.tile([P, CT], F32)
            nc.sync.dma_start(out=bq_sb, in_=bq_d[:])
            bk_sb = big.tile([P, CT], F32)
            nc.sync.dma_start(out=bk_sb, in_=bk_d[:])
            bv_sb = big.tile([1, C], BF16)
            nc.sync.dma_start(out=bv_sb, in_=bv_d[:])
            bo_sb = big.tile([1, C], BF16)
            nc.sync.dma_start(out=bo_sb, in_=bo_d[:])
            gam_sb = big.tile([P, CT], F32)
            nc.sync.dma_start(out=gam_sb, in_=gam_d[:])
            bet_sb = big.tile([P, CT], F32)
            nc.sync.dma_start(out=bet_sb, in_=bet_d[:])
            gsel_sb = big.tile([P, CT, G], F32)
            nc.sync.dma_start(out=gsel_sb, in_=gsel_d[:])
            gbc_sb = big.tile([G, CT, P], F32)
            nc.sync.dma_start(out=gbc_sb, in_=gbc_d[:])

            # Trigger the exp table load early so it is off the critical path.
            dummy = big.tile([1, 1], F32)
            nc.scalar.activation(out=dummy, in_=ones_1_128[0:1, 0:1], func=AF.Exp)

            # ---- GroupNorm statistics ----
            # per-channel mean/var via bn_stats, then group-aggregate on PE.
            psg = psum_tile()  # [0:32, 0, 0:2] = [mean_g, E2_g]
            for t in range(CT):
                st6 = work.tile([P, 2, 6], F32, tag="st6")
                for j in range(2):
                    nc.vector.bn_stats(
                        out=st6[:, j, :], in_=x_sb[:, t, 512 * j:512 * (j + 1)]
                    )
                mv = work.tile([P, 2], F32, tag="mv")
                nc.vector.bn_aggr(out=mv, in_=st6)
                # rhs2 = [mean_c, var_c + mean_c^2]
                sq = work.tile([P, 1], F32, tag="sq")
                nc.vector.tensor_mul(sq, mv[:, 0:1], mv[:, 0:1])
                rhs2 = work.tile([P, 2], F32, tag="rhs2")
                nc.vector.tensor_copy(rhs2[:, 0:1], mv[:, 0:1])
                nc.vector.tensor_add(rhs2[:, 1:2], mv[:, 1:2], sq)
                nc.tensor.matmul(
                    psg[0:G, 0, 0:2],
                    lhsT=gsel_sb[:, t, :],
                    rhs=rhs2,
                    start=(t == 0),
                    stop=(t == CT - 1),
                )

            # stats2 = [mean_g, rstd_g] in SBUF (32, 2)
            stats2 = big.tile([G, 2], F32)
            nc.vector.tensor_copy(stats2[:, 0:1], psg[0:G, 0, 0:1])
            sqg = scal.tile([G, 1], F32, tag="sqg")
            nc.vector.tensor_mul(sqg, stats2[:, 0:1], stats2[:, 0:1])
            varg = scal.tile([G, 1], F32, tag="varg")
            nc.vector.tensor_sub(varg, psg[0:G, 0, 1:2], sqg)
            nc.vector.tensor_scalar(
                out=varg, in0=varg, scalar1=EPS, scalar2=None, op0=ALU.add
            )
            # rstd = rsqrt(varg) via bit-trick + 3 Newton iterations (all DVE)
            yv = scal.tile([G, 1], F32, tag="yv")
            tI = scal.tile([G, 1], I32, tag="tI")
            nc.vector.tensor_scalar(
                out=tI, in0=varg.bitcast(I32), scalar1=1, scalar2=None,
                op0=ALU.logical_shift_right,
            )
            nc.vector.tensor_scalar(
                out=yv.bitcast(I32), in0=tI, scalar1=-1, scalar2=0x5F3759DF,
                op0=ALU.mult, op1=ALU.add,
            )
            for _ in range(2):
                t1 = scal.tile([G, 1], F32, tag="t1")
                nc.vector.tensor_mul(t1, yv, yv)
                nc.vector.tensor_mul(t1, t1, varg)
                nc.vector.tensor_scalar(
                    out=t1, in0=t1, scalar1=-0.5, scalar2=1.5,
                    op0=ALU.mult, op1=ALU.add,
                )
                nc.vector.tensor_mul(yv, yv, t1)
            nc.vector.tensor_copy(stats2[:, 1:2], yv)

            # ---- normalize: xn = x*A + B (per channel) ----
            xn_sb = big.tile([P, CT, L], BF16)
            for t in range(CT):
                psb = psum_tile()
                nc.tensor.matmul(
                    psb[0:P, 0, 0:2], lhsT=gbc_sb[:, t, :], rhs=stats2,
                    start=True, stop=True,
                )
                a_t = scal.tile([P, 1], F32, tag="a_t")
                nc.vector.tensor_mul(a_t, psb[0:P, 0, 1:2], gam_sb[:, t:t + 1])
                tmp = scal.tile([P, 1], F32, tag="tmp")
                nc.vector.tensor_mul(tmp, psb[0:P, 0, 0:1], a_t)
                b_t = scal.tile([P, 1], F32, tag="b_t")
                nc.vector.tensor_sub(b_t, bet_sb[:, t:t + 1], tmp)
                nc.vector.tensor_scalar(
                    out=xn_sb[:, t, :], in0=x_sb[:, t, :],
                    scalar1=a_t, scalar2=b_t, op0=ALU.mult, op1=ALU.add,
                )

            if debug:
                nc.sync.dma_start(out=dbg_stats_d[:], in_=stats2)
                nc.sync.dma_start(out=dbg_xn_d[:], in_=xn_sb)

            # ---- projections + attention, interleaved so ACT starts early.
            # q/k output-channel tile j feeds exactly head pair j, so emit
            # q_j,k_j then pair-j dots/exp; vT after pair-0 dots (PE fills it
            # in while ACT churns pair-0 exps).
            q_sb = big.tile([P, CT, L], BF16)
            k_sb = big.tile([P, CT, L], BF16)
            vT_sb = big.tile([P, ST, NH * 80], FP8)
            attn_sb = big.tile([DH, CT, 2, L], FP8)
            vT_h = vT_sb.rearrange("p s (h c) -> p s h c", c=80)

            def emit_qk(j):
                for h2 in range(TH):
                    pq = psum_tile()
                    for ct in range(CT):
                        nc.tensor.matmul(
                            pq[:, 0, :],
                            lhsT=wqt_sb[:, ct, 128 * j:128 * (j + 1)],
                            rhs=xn_sb[:, ct, 512 * h2:512 * (h2 + 1)],
                            start=(ct == 0), stop=(ct == CT - 1),
                        )
                    nc.vector.tensor_scalar(
                        out=q_sb[:, j, 512 * h2:512 * (h2 + 1)], in0=pq[:, 0, :],
                        scalar1=bq_sb[:, j:j + 1], scalar2=None, op0=ALU.add,
                    )
                    pk = psum_tile()
                    for ct in range(CT):
                        nc.tensor.matmul(
                            pk[:, 0, :],
                            lhsT=wkt_sb[:, ct, 128 * j:128 * (j + 1)],
                            rhs=xn_sb[:, ct, 512 * h2:512 * (h2 + 1)],
                            start=(ct == 0), stop=(ct == CT - 1),
                        )
                    nc.vector.tensor_scalar(
                        out=k_sb[:, j, 512 * h2:512 * (h2 + 1)], in0=pk[:, 0, :],
                        scalar1=bk_sb[:, j:j + 1], scalar2=None, op0=ALU.add,
                    )

            def emit_vt():
                # vT: (l, o) fp8; head stride 80 (64 v + ones@64 + pad) keeps
                # DoubleRow APs 16-aligned.
                ones_cols = vT_sb.rearrange(
                    "p s (h c) -> p s h c", c=80
                )[:, :, :, 64:65]
                nc.vector.memset(ones_cols, 1.0)
                for lt in range(ST):
                    pv = psum_tile()
                    for ct in range(CT):
                        nc.tensor.matmul(
                            pv[:, 0, :],
                            lhsT=xn_sb[:, ct, 128 * lt:128 * (lt + 1)],
                            rhs=wvt_sb[:, ct, :],
                            start=(ct == 0), stop=False,
                        )
                    nc.tensor.matmul(
                        pv[:, 0, :], lhsT=ones_1_128, rhs=bv_sb,
                        start=False, stop=True,
                    )
                    dst = vT_sb.rearrange(
                        "p s (h c) -> p s h c", c=80
                    )[:, lt, :, 0:64]
                    src = pv[:, 0, :].rearrange("p (h c) -> p h c", c=64)
                    nc.vector.tensor_copy(dst, src)

            def emit_dots(pr):
                pA = pbuf.tile([P, ST, L], FP8, tag="p", name=f"pA{pr}")
                pB = pbuf.tile([P, ST, L], FP8, tag="p", name=f"pB{pr}")
                for j in range(ST):
                    _psn[0] += 1
                    pdA = psp.tile([P, 2, 512], F32, tag="dots", name=f"pd{_psn[0]}")
                    _psn[0] += 1
                    pdB = psp.tile([P, 2, 512], F32, tag="dots", name=f"pd{_psn[0]}")
                    for th in range(TH):
                        nc.tensor.matmul(
                            pdA[:, th, :],
                            lhsT=k_sb[0:64, pr, 128 * j:128 * (j + 1)],
                            rhs=q_sb[0:64, pr, 512 * th:512 * (th + 1)],
                            start=True, stop=True,
                        )
                        nc.tensor.matmul(
                            pdB[:, th, :],
                            lhsT=k_sb[64:128, pr, 128 * j:128 * (j + 1)],
                            rhs=q_sb[64:128, pr, 512 * th:512 * (th + 1)],
                            start=True, stop=True,
                        )
                    # p~ = 128*exp(dots/64) in fp8e4 (~[110,150], e4m3 max
                    # 240); the 128 cancels between numerator and rowsum.
                    nc.scalar.activation(
                        out=pA[:, j, :].rearrange("p (a b) -> p a b", a=2),
                        in_=pdA[:, :, :], func=AF.Exp, scale=1.0 / 64.0,
                        bias=ln256_b,
                    )
                    nc.scalar.activation(
                        out=pB[:, j, :].rearrange("p (a b) -> p a b", a=2),
                        in_=pdB[:, :, :], func=AF.Exp, scale=1.0 / 64.0,
                        bias=ln256_b,
                    )
                if debug and pr == 0:
                    nc.sync.dma_start(out=dbg_p_d[0], in_=pA)
                    nc.sync.dma_start(out=dbg_p_d[1], in_=pB)
                return pA, pB

            def emit_attn(pr, pA, pB):
                hA, hB = 2 * pr, 2 * pr + 1
                for th in range(TH):
                    # DoubleRow forbids col tile_position, so all four
                    # matmuls run at (0,0); the A/B pair shares one 2-bank
                    # slot (different banks).  Head B lands at partitions
                    # 0-63 and is DMA-moved to 64-127.
                    pavAB = psum_tile()
                    prsAB = psum_tile()
                    pavA = pavAB[0:64, 0, :]
                    pavB = pavAB[0:64, 1, :]
                    prsA = prsAB[0:1, 0, :]
                    prsB = prsAB[0:1, 1, :]
                    tsl = slice(512 * th, 512 * (th + 1))
                    DR = mybir.MatmulPerfMode.DoubleRow
                    for j in range(0, ST, 2):
                        st_, sp_ = (j == 0), (j == ST - 2)
                        nc.tensor.matmul(
                            pavA, lhsT=vT_h[:, j:j + 2, hA, 0:64],
                            rhs=pA[:, j:j + 2, tsl], start=st_, stop=sp_,
                            perf_mode=DR,
                        )
                        nc.tensor.matmul(
                            pavB, lhsT=vT_h[:, j:j + 2, hB, 0:64],
                            rhs=pB[:, j:j + 2, tsl], start=st_, stop=sp_,
                            perf_mode=DR,
                        )
                        nc.tensor.matmul(
                            prsA, lhsT=vT_h[:, j:j + 2, hA, 64:65],
                            rhs=pA[:, j:j + 2, tsl], start=st_, stop=sp_,
                            perf_mode=DR,
                        )
                        nc.tensor.matmul(
                            prsB, lhsT=vT_h[:, j:j + 2, hB, 64:65],
                            rhs=pB[:, j:j + 2, tsl], start=st_, stop=sp_,
                            perf_mode=DR,
                        )
                    recA = bcp.tile([1, 512], F32, tag="recA")
                    recB = bcp.tile([1, 512], F32, tag="recB")
                    nc.vector.reciprocal(recA, prsA)
                    nc.vector.reciprocal(recB, prsB)
                    # Broadcast rec across partitions on GpSimd (base-0 APs
                    # only -- nonzero partition bases are broken in ucode).
                    bc = bcp.tile([64, 512], F32, tag="bc")
                    nc.gpsimd.partition_broadcast(bc[0:64, :], recA, channels=64)
                    bc2 = bcp.tile([64, 512], F32, tag="bc2")
                    nc.gpsimd.partition_broadcast(bc2[0:64, :], recB, channels=64)
                    if debug and pr == 0 and th == 0:
                        nc.sync.dma_start(out=dbg_rs_d[0:1, :], in_=recA)
                        nc.sync.dma_start(out=dbg_rs_d[1:2, :], in_=recB)
                        pavc = bcp.tile([P, 512], F32, tag="pavc")
                        nc.vector.tensor_copy(pavc[0:64, :], pavA)
                        nc.vector.tensor_copy(pavc[64:128, :], pavB)
                        nc.sync.dma_start(out=dbg_pav_d[:], in_=pavc)
                    nc.vector.tensor_mul(
                        attn_sb[0:DH, pr, 0, tsl], pavA, bc[0:64, :]
                    )
                    nc.vector.tensor_mul(
                        attn_sb[0:DH, pr, 1, tsl], pavB, bc2[0:64, :]
                    )

            emit_qk(0)
            prev = emit_dots(0)
            emit_vt()
            emit_qk(1)
            for pr in range(1, CT):
                cur = emit_dots(pr)
                emit_attn(pr - 1, *prev)
                prev = cur
                if pr + 1 < CT:
                    emit_qk(pr + 1)
            emit_attn(CT - 1, *prev)

            if debug:
                nc.sync.dma_start(out=dbg_q_d[:], in_=q_sb)
                nc.sync.dma_start(out=dbg_k_d[:], in_=k_sb)
                nc.sync.dma_start(out=dbg_vt_d[:], in_=vT_sb)
                nc.sync.dma_start(out=dbg_attn_d[:], in_=attn_sb)

            # ---- output projection + residual ----
            for j in range(CT):
                for h2 in range(TH):
                    po = psum_tile()
                    for dt_ in range(CT):
                        nc.tensor.matmul(
                            po[:, 0, :],
                            lhsT=wot_sb[0:DH, dt_, :, 128 * j:128 * (j + 1)],
                            rhs=attn_sb[0:DH, dt_, :, 512 * h2:512 * (h2 + 1)],
                            start=(dt_ == 0), stop=False,
                            perf_mode=mybir.MatmulPerfMode.DoubleRow,
                        )
                    nc.tensor.matmul(
                        po[:, 0, :], lhsT=bo_sb[:, 128 * j:128 * (j + 1)],
                        rhs=ones_1_512, start=False, stop=True,
                    )
                    ytile = yp.tile([P, 512], F32, tag="y")
                    nc.vector.tensor_add(
                        ytile, po[:, 0, :], x_sb[:, j, 512 * h2:512 * (h2 + 1)]
                    )
                    nc.sync.dma_start(
                        out=y_d[:, j, 512 * h2:512 * (h2 + 1)], in_=ytile
                    )

    return nc


def _ctile(a):
    """(512, X) -> (128, 4, X) channel-tile layout."""
    return np.ascontiguousarray(
        a.reshape(4, 128, *a.shape[1:]).transpose(1, 0, *range(2, a.ndim + 1))
    )


def prep_consts(gamma, beta, Wq, bq, Wkv, bkv, Wo, bo):
    grp = np.arange(C) // GS
    gsel = (grp[:, None] == np.arange(G)[None, :]).astype(np.float32) / GS
    gbc = (np.arange(G)[:, None] == grp[None, :]).astype(np.float32)
    consts = {
        "wqt": _ctile(np.ascontiguousarray(Wq.T)).astype(NP_BF16),
        "wkt": _ctile(np.ascontiguousarray(Wkv[:C].T)).astype(NP_BF16),
        "wvt": _ctile(np.ascontiguousarray(Wkv[C:].T)).astype(NP_BF16),
        "wot": np.ascontiguousarray(
            Wo.T.reshape(CT, 2, DH, C).transpose(2, 0, 1, 3)
        ).astype(NP_FP8),
        "bq": np.ascontiguousarray(bq.reshape(4, 128).T).astype(np.float32),
        "bk": np.ascontiguousarray(bkv[:C].reshape(4, 128).T).astype(np.float32),
        "bv": bkv[C:].reshape(1, C).astype(NP_BF16),
        "bo": bo.reshape(1, C).astype(NP_BF16),
        "gamma": np.ascontiguousarray(gamma.reshape(4, 128).T).astype(np.float32),
        "beta": np.ascontiguousarray(beta.reshape(4, 128).T).astype(np.float32),
        "gsel": np.ascontiguousarray(gsel.reshape(4, 128, G).transpose(1, 0, 2)),
        "gbc": np.ascontiguousarray(gbc.reshape(G, 4, 128)),
    }
    return consts


def prep_x(x):
    """(8, 512, 32, 32) -> list of per-core (128, 4, 1024) f32."""
    xf = np.asarray(x, dtype=np.float32).reshape(8, C, L)
    return [_ctile(xf[i]) for i in range(8)]


def unprep_y(ys):
    """list of per-core (128, 4, 1024) -> (8, 512, 32, 32)."""
    out = np.empty((8, C, 32, 32), dtype=np.float32)
    for i, yi in enumerate(ys):
        out[i] = yi.transpose(1, 0, 2).reshape(C, 32, 32)
    return out


_NC_CACHE = None


def kernel(x, gamma, beta, Wq, bq, Wkv, bkv, Wo, bo):
    global _NC_CACHE
    from concourse.bass_utils import run_bass_kernel_spmd

    if _NC_CACHE is None:
        _NC_CACHE = build_nc()
        _NC_CACHE.finalize()
    nc = _NC_CACHE

    consts = prep_consts(
        np.asarray(gamma, np.float32), np.asarray(beta, np.float32),
        np.asarray(Wq, np.float32), np.asarray(bq, np.float32),
        np.asarray(Wkv, np.float32), np.asarray(bkv, np.float32),
        np.asarray(Wo, np.float32), np.asarray(bo, np.float32),
    )
    xs = prep_x(x)
    in_maps = [{**consts, "x": xs[i]} for i in range(8)]
    res = run_bass_kernel_spmd(nc, in_maps, core_ids=list(range(8)))
    return unprep_y([r["y"] for r in res.results])
